# revision 10
# baseline (speedup 1.0000x reference)
"""Trainium2 Bass kernel for a 2-layer LSTM (B=128, T=512, V=128, H=512).

Strategy (data-parallel over batch, 8 cores, B_local=16 per core):
  Phase A: layer-0 recurrence over T steps. Per step the gates psum
           (16, 4H) accumulates x_t^T-stationary @ Wx0 plus 4 h-chunk
           matmuls against Wh0 (weights are the moving operand).
           h0_t is transposed back (PE transpose) for the next step's
           stationary operand and saved to DRAM for phase B.
  Phase B: G1 = H0 @ Wx1 (+ b1) as a batched matmul over all T.
  Phase C: layer-1 recurrence; per step psum is seeded with G1_t via a
           k=16 identity matmul, then 4 h-chunk matmuls against Wh1.
           h1_t streams to the output.
All compute fp32.
"""
import sys

import numpy as np

for _p in ("/opt/trn_rl_repo", "/root/.axon_site/_ro/trn_rl_repo"):
    if _p not in sys.path:
        sys.path.insert(0, _p)

import concourse.bacc as bacc
import concourse.mybir as mybir
import concourse.tile as tile
from concourse.bass_utils import run_bass_kernel_spmd

B, T, V, H = 128, 512, 128, 512
G4 = 4 * H            # 2048 gate columns, reordered [i|f|o|g] (one sigmoid span)
NCORES = 8
BL = B // NCORES      # 16 batch rows per core
KH = H // 128         # 4 contraction chunks for H
NBANK = 4
BANK = G4 // NBANK    # 512 = one psum bank (fp32)
MROWS = 128           # phase-B output rows per chunk
STEPS_PER_M = MROWS // BL  # 8 time steps per phase-B chunk

F32 = mybir.dt.float32
F32R = mybir.dt.float32r   # same 4-byte layout; single-pass PE matmul (4x faster)
AF = mybir.ActivationFunctionType

RUN_KWARGS: dict = {}   # test.py can set {"trace": True}
_BUILD_CACHE: dict = {}


def _build(t_steps: int, has_b0: bool, has_b1: bool):
    nc = bacc.Bacc("TRN2", target_bir_lowering=False, debug=False,
                   num_devices=NCORES)

    # ---- I/O ----
    xt = nc.dram_tensor("xt", [t_steps, V, BL], F32R, kind="ExternalInput")
    w0 = nc.dram_tensor("w0", [V, 5 * G4], F32R, kind="ExternalInput")
    w1x = nc.dram_tensor("w1x", [128, KH * G4], F32R, kind="ExternalInput")
    w1h = nc.dram_tensor("w1h", [128, KH * G4], F32R, kind="ExternalInput")
    eye = nc.dram_tensor("eye", [BL, BL], F32R, kind="ExternalInput")
    b0r = nc.dram_tensor("b0r", [1, G4], F32R, kind="ExternalInput")
    b1r = nc.dram_tensor("b1r", [1, G4], F32R, kind="ExternalInput")
    onesr = nc.dram_tensor("onesr", [1, 128], F32R, kind="ExternalInput")
    h0t0 = nc.dram_tensor("h0t0", [128, KH * BL], F32R, kind="ExternalInput")
    h1t0 = nc.dram_tensor("h1t0", [128, KH * BL], F32R, kind="ExternalInput")
    c00 = nc.dram_tensor("c00", [BL, H], F32, kind="ExternalInput")
    c10 = nc.dram_tensor("c10", [BL, H], F32, kind="ExternalInput")

    outs = nc.dram_tensor("outs", [BL, t_steps, H], F32, kind="ExternalOutput")
    hf = nc.dram_tensor("hf", [2, BL, H], F32, kind="ExternalOutput")
    cf = nc.dram_tensor("cf", [2, BL, H], F32, kind="ExternalOutput")

    with tile.TileContext(nc) as tc:
        with (
            tc.tile_pool(name="dram", bufs=1, space="DRAM") as dram,
            tc.tile_pool(name="singles", bufs=1) as singles,
            tc.tile_pool(name="xp", bufs=3) as xp,
            tc.tile_pool(name="rgp", bufs=2) as rgp,
            tc.tile_pool(name="psg", bufs=1, space="PSUM") as psg,
            tc.tile_pool(name="pst", bufs=2, space="PSUM") as pst,
            tc.tile_pool(name="actp", bufs=2) as actp,
            tc.tile_pool(name="tmp", bufs=2) as tmp,
            tc.tile_pool(name="cstp", bufs=2) as cstp,
            tc.tile_pool(name="hp", bufs=2) as hp,
            tc.tile_pool(name="htp", bufs=3) as htp,
            tc.tile_pool(name="statp", bufs=3) as statp,
            tc.tile_pool(name="gcp", bufs=1) as gcp,
        ):
            # persistent DRAM intermediates
            H0T = dram.tile([KH, 128, t_steps, BL], F32R)
            G1 = dram.tile([t_steps, BL, G4], F32R)

            # resident weights / constants
            w0s = singles.tile([V, 5 * G4], F32R)
            nc.sync.dma_start(w0s[:], w0[:])
            w1xs = singles.tile([128, KH * G4], F32R)
            nc.sync.dma_start(w1xs[:], w1x[:])
            w1hs = singles.tile([128, KH * G4], F32R)
            nc.sync.dma_start(w1hs[:], w1h[:])
            eye_s = singles.tile([BL, BL], F32R)
            nc.sync.dma_start(eye_s[:], eye[:])
            eye_f = singles.tile([BL, BL], F32)
            nc.sync.dma_start(eye_f[:], eye[:].bitcast(F32))
            ones1 = singles.tile([1, 128], F32R)
            nc.sync.dma_start(ones1[:], onesr[:])
            b0s = singles.tile([1, G4], F32R)
            nc.sync.dma_start(b0s[:], b0r[:])
            b1s = singles.tile([1, G4], F32R)
            nc.sync.dma_start(b1s[:], b1r[:])

            def mm(pb, lhsT, rhs, **kw):
                nc.tensor.matmul(pb, lhsT, rhs, **kw)

            def cell_tail(ps, cst, layer, t):
                """activations + state update; returns (cnew, hnew, hTn)."""
                sifo = actp.tile([BL, 3 * BANK], F32, tag="sifo")
                nc.scalar.activation(sifo[:], ps[:, 0:3 * BANK], AF.Sigmoid)
                tg = actp.tile([BL, BANK], F32, tag="tg")
                nc.scalar.activation(tg[:], ps[:, 3 * BANK:4 * BANK], AF.Tanh)

                t1 = tmp.tile([BL, H], F32, tag="t1")
                nc.vector.tensor_mul(t1[:], sifo[:, 0:BANK], tg[:])
                t2 = tmp.tile([BL, H], F32, tag="t2")
                nc.vector.tensor_mul(t2[:], sifo[:, BANK:2 * BANK], cst[:])
                cnew = cstp.tile([BL, H], F32, tag=f"c{layer}")
                nc.vector.tensor_add(cnew[:], t1[:], t2[:])
                tc_ = tmp.tile([BL, H], F32, tag="tc")
                nc.scalar.activation(tc_[:], cnew[:], AF.Tanh)
                hnew = hp.tile([BL, H], F32, tag=f"h{layer}")
                nc.vector.tensor_mul(hnew[:], sifo[:, 2 * BANK:3 * BANK],
                                     tc_[:])

                pt = pst.tile([128, KH * BL], F32, tag="pt")
                for j in range(KH):
                    nc.tensor.transpose(
                        pt[:, j * BL:(j + 1) * BL],
                        hnew[:, j * 128:(j + 1) * 128],
                        eye_f[:],
                    )
                hTn = htp.tile([128, KH * BL], F32R, tag=f"ht{layer}")
                nc.vector.tensor_copy(hTn[:], pt[:])
                return cnew, hnew, hTn

            # ---------------- Phase A: layer 0 ----------------
            hT = htp.tile([128, KH * BL], F32R, tag="ht0")
            nc.sync.dma_start(hT[:], h0t0[:])
            cst = cstp.tile([BL, H], F32, tag="c0")
            nc.sync.dma_start(cst[:], c00[:])

            for t in range(t_steps):
                x_t = xp.tile([V, BL], F32R, tag="x")
                nc.sync.dma_start(x_t[:], xt[t])

                ps = psg.tile([BL, G4], F32, tag="ps")
                for bank in range(NBANK):
                    pb = ps[:, bank * BANK:(bank + 1) * BANK]
                    mm(pb, x_t[:], w0s[:, bank * BANK:bank * BANK + BANK],
                       start=True, stop=False)
                    for j in range(KH):
                        off = (1 + j) * G4 + bank * BANK
                        last = (j == KH - 1) and not has_b0
                        mm(pb, hT[:, j * BL:(j + 1) * BL],
                           w0s[:, off:off + BANK], start=False, stop=last)
                    if has_b0:
                        mm(pb, ones1[:, 0:BL],
                           b0s[:, bank * BANK:(bank + 1) * BANK],
                           start=False, stop=True)

                cst, hnew, hT = cell_tail(ps, cst, 0, t)
                nc.sync.dma_start(
                    H0T[:, :, t, :].rearrange("k p b -> p k b"), hT[:])
                if t == t_steps - 1:
                    nc.sync.dma_start(hf[0], hnew[:])
                    nc.sync.dma_start(cf[0], cst[:])

            # ---------------- Phase B: G1 = H0 @ Wx1 (+ b1) ----------------
            for mc in range(t_steps // STEPS_PER_M):
                t0 = mc * STEPS_PER_M
                stat = statp.tile([128, KH * MROWS], F32R, tag="stat")
                for j in range(KH):
                    nc.sync.dma_start(
                        stat[:, j * MROWS:(j + 1) * MROWS],
                        H0T[j, :, t0:t0 + STEPS_PER_M, :])
                psb = psg.tile([128, G4], F32, tag="ps")
                for bank in range(NBANK):
                    pb = psb[:, bank * BANK:(bank + 1) * BANK]
                    for j in range(KH):
                        off = j * G4 + bank * BANK
                        last = (j == KH - 1) and not has_b1
                        mm(pb, stat[:, j * MROWS:(j + 1) * MROWS],
                           w1xs[:, off:off + BANK],
                           start=(j == 0), stop=last)
                    if has_b1:
                        mm(pb, ones1[:],
                           b1s[:, bank * BANK:(bank + 1) * BANK],
                           start=False, stop=True)
                gt = gcp.tile([128, G4], F32R, tag="gt")
                nc.vector.tensor_copy(gt[:], psb[:])
                nc.sync.dma_start(G1[t0:t0 + STEPS_PER_M], gt[:])

            # ---------------- Phase C: layer 1 ----------------
            hT = htp.tile([128, KH * BL], F32R, tag="ht1")
            nc.sync.dma_start(hT[:], h1t0[:])
            cst = cstp.tile([BL, H], F32, tag="c1")
            nc.sync.dma_start(cst[:], c10[:])

            for t in range(t_steps):
                rg = rgp.tile([BL, G4], F32R, tag="rg")
                nc.sync.dma_start(rg[:], G1[t])

                ps = psg.tile([BL, G4], F32, tag="ps")
                for bank in range(NBANK):
                    pb = ps[:, bank * BANK:(bank + 1) * BANK]
                    mm(pb, eye_s[:], rg[:, bank * BANK:(bank + 1) * BANK],
                       start=True, stop=False)
                    for j in range(KH):
                        off = j * G4 + bank * BANK
                        mm(pb, hT[:, j * BL:(j + 1) * BL],
                           w1hs[:, off:off + BANK],
                           start=False, stop=(j == KH - 1))

                cst, hnew, hT = cell_tail(ps, cst, 1, t)
                nc.sync.dma_start(outs[:, t, :], hnew[:])
                if t == t_steps - 1:
                    nc.sync.dma_start(hf[1], hnew[:])
                    nc.sync.dma_start(cf[1], cst[:])

    nc.compile()
    return nc


def _get(t_steps: int, has_b0: bool, has_b1: bool):
    key = (t_steps, has_b0, has_b1)
    if key not in _BUILD_CACHE:
        _BUILD_CACHE[key] = _build(t_steps, has_b0, has_b1)
    return _BUILD_CACHE[key]


def _hT_chunks(hmat: np.ndarray) -> np.ndarray:
    """(BL, H) -> (128, KH*BL) stationary layout (chunk-major columns)."""
    return np.ascontiguousarray(
        hmat.T.reshape(KH, 128, BL).transpose(1, 0, 2).reshape(128, KH * BL))


def kernel(x, h, c, Wx0, Wh0, b0, Wx1, Wh1, b1, t_steps: int = T):
    x = np.asarray(x, np.float32)
    h = np.asarray(h, np.float32)
    c = np.asarray(c, np.float32)
    Wx0 = np.asarray(Wx0, np.float32)
    Wh0 = np.asarray(Wh0, np.float32)
    b0 = np.asarray(b0, np.float32)
    Wx1 = np.asarray(Wx1, np.float32)
    Wh1 = np.asarray(Wh1, np.float32)
    b1 = np.asarray(b1, np.float32)

    has_b0 = bool(np.any(b0))
    has_b1 = bool(np.any(b1))
    nc = _get(t_steps, has_b0, has_b1)

    def _gperm(w):
        # reorder gate column groups [i f g o] -> [i f o g]
        i_, f_, g_, o_ = np.split(w, 4, axis=-1)
        return np.concatenate([i_, f_, o_, g_], axis=-1)

    Wx0, Wh0, Wx1, Wh1 = map(_gperm, (Wx0, Wh0, Wx1, Wh1))
    b0, b1 = _gperm(b0), _gperm(b1)
    w0cat = np.concatenate(
        [Wx0[None], Wh0.reshape(KH, 128, G4)], axis=0)  # (5,128,G4)
    w0_in = np.ascontiguousarray(
        w0cat.transpose(1, 0, 2).reshape(V, 5 * G4))
    w1x_in = np.ascontiguousarray(
        Wx1.reshape(KH, 128, G4).transpose(1, 0, 2).reshape(128, KH * G4))
    w1h_in = np.ascontiguousarray(
        Wh1.reshape(KH, 128, G4).transpose(1, 0, 2).reshape(128, KH * G4))
    eye_in = np.eye(BL, dtype=np.float32)
    b0_in = np.ascontiguousarray(b0[None])
    b1_in = np.ascontiguousarray(b1[None])

    in_maps = []
    for i in range(NCORES):
        s = slice(i * BL, (i + 1) * BL)
        in_maps.append({
            "xt": np.ascontiguousarray(x[s, :t_steps].transpose(1, 2, 0)),
            "w0": w0_in, "w1x": w1x_in, "w1h": w1h_in,
            "eye": eye_in, "b0r": b0_in, "b1r": b1_in,
            "onesr": np.ones((1, 128), np.float32),
            "h0t0": _hT_chunks(h[0, s]),
            "h1t0": _hT_chunks(h[1, s]),
            "c00": np.ascontiguousarray(c[0, s]),
            "c10": np.ascontiguousarray(c[1, s]),
        })

    res = run_bass_kernel_spmd(nc, in_maps, core_ids=list(range(NCORES)),
                               **RUN_KWARGS)
    kernel.last_results = res

    outs = np.empty((B, t_steps, H), np.float32)
    h_f = np.empty((2, B, H), np.float32)
    c_f = np.empty((2, B, H), np.float32)
    for i in range(NCORES):
        s = slice(i * BL, (i + 1) * BL)
        outs[s] = res.results[i]["outs"]
        h_f[:, s] = res.results[i]["hf"]
        c_f[:, s] = res.results[i]["cf"]
    return outs, h_f, c_f


# revision 13
# speedup vs baseline: 1.0373x; 1.0373x over previous
"""Trainium2 Bass kernel for a 2-layer LSTM (B=128, T=512, V=128, H=512).

Data-parallel over batch (8 cores, B_local=16). Wavefront schedule:
layer-0 steps of time-chunk mc, then the batched H0@Wx1 transform for
that chunk, then layer-1 steps of chunk mc-1 — all interleaved so each
layer's serial activation tail hides under the other layer's matmuls.
Gate matmuls run as float32r (single-pass PE). All staging in SBUF —
no DRAM bounce buffers.

Per step (banks ordered [g|i|f|o]):
  psum(16,2048) = x_t^T @ Wx0 (L0) or G1_t via k=16 identity (L1),
                  += 4 h-chunk matmuls against Wh (moving weights)
  tanh/sigmoid on ACT, c/h update on DVE, h transposed back via PE
  transpose into the tail of the o-bank psum, copied to the chunk's
  hT tile (stationary for the next step and the batched transform).
"""
import sys

import numpy as np

for _p in ("/opt/trn_rl_repo", "/root/.axon_site/_ro/trn_rl_repo"):
    if _p not in sys.path:
        sys.path.insert(0, _p)

import concourse.bacc as bacc
import concourse.mybir as mybir
import concourse.tile as tile
from concourse.bass_utils import run_bass_kernel_spmd

B, T, V, H = 128, 512, 128, 512
G4 = 4 * H            # 2048 gate columns, order [g|i|f|o]
NCORES = 8
BL = B // NCORES      # 16 batch rows per core
KH = H // 128         # 4 contraction chunks for H
NBANK = 4
BANK = G4 // NBANK    # 512 = one psum bank (fp32)
SPC = 8               # time steps per chunk
SLOT = KH * BL        # 64 cols of hT per step

F32 = mybir.dt.float32
F32R = mybir.dt.float32r
AF = mybir.ActivationFunctionType

RUN_KWARGS: dict = {}
_BUILD_CACHE: dict = {}


def _build(t_steps: int, has_b0: bool, has_b1: bool):
    assert t_steps % SPC == 0
    nch = t_steps // SPC
    nc = bacc.Bacc("TRN2", target_bir_lowering=False, debug=False,
                   num_devices=NCORES)

    xt = nc.dram_tensor("xt", [t_steps, V, BL], F32R, kind="ExternalInput")
    w0 = nc.dram_tensor("w0", [V, 5 * G4], F32R, kind="ExternalInput")
    w1x = nc.dram_tensor("w1x", [128, KH * G4], F32R, kind="ExternalInput")
    w1h = nc.dram_tensor("w1h", [128, KH * G4], F32R, kind="ExternalInput")
    eye = nc.dram_tensor("eye", [BL, BL], F32R, kind="ExternalInput")
    b0r = nc.dram_tensor("b0r", [1, G4], F32R, kind="ExternalInput")
    b1r = nc.dram_tensor("b1r", [1, G4], F32R, kind="ExternalInput")
    onesr = nc.dram_tensor("onesr", [1, 128], F32R, kind="ExternalInput")
    h0t0 = nc.dram_tensor("h0t0", [128, SLOT], F32R, kind="ExternalInput")
    h1t0 = nc.dram_tensor("h1t0", [128, SLOT], F32R, kind="ExternalInput")
    c00 = nc.dram_tensor("c00", [BL, H], F32, kind="ExternalInput")
    c10 = nc.dram_tensor("c10", [BL, H], F32, kind="ExternalInput")

    outs = nc.dram_tensor("outs", [BL, t_steps, H], F32, kind="ExternalOutput")
    hf = nc.dram_tensor("hf", [2, BL, H], F32, kind="ExternalOutput")
    cf = nc.dram_tensor("cf", [2, BL, H], F32, kind="ExternalOutput")

    with tile.TileContext(nc) as tc:
        with (
            tc.tile_pool(name="singles", bufs=1) as singles,
            tc.tile_pool(name="xp", bufs=3) as xp,
            tc.tile_pool(name="rgp", bufs=2) as rgp,
            tc.tile_pool(name="psum", bufs=1, space="PSUM") as psum,
            tc.tile_pool(name="actp", bufs=2) as actp,
            tc.tile_pool(name="tmp", bufs=3) as tmp,
            tc.tile_pool(name="cstp", bufs=2) as cstp,
            tc.tile_pool(name="hp", bufs=3) as hp,
            tc.tile_pool(name="htcp", bufs=2) as htcp,
            tc.tile_pool(name="gtp", bufs=2) as gtp,
        ):
            w0s = singles.tile([V, 5 * G4], F32R)
            nc.sync.dma_start(w0s[:], w0[:])
            w1xs = singles.tile([128, KH * G4], F32R)
            nc.sync.dma_start(w1xs[:], w1x[:])
            w1hs = singles.tile([128, KH * G4], F32R)
            nc.sync.dma_start(w1hs[:], w1h[:])
            eye_s = singles.tile([BL, BL], F32R)
            nc.sync.dma_start(eye_s[:], eye[:])
            eye_f = singles.tile([BL, BL], F32)
            nc.sync.dma_start(eye_f[:], eye[:].bitcast(F32))
            ones1 = None
            if has_b0 or has_b1:
                ones1 = singles.tile([1, 128], F32R)
                nc.sync.dma_start(ones1[:], onesr[:])
            b0s = b1s = None
            if has_b0:
                b0s = singles.tile([1, G4], F32R)
                nc.sync.dma_start(b0s[:], b0r[:])
            if has_b1:
                b1s = singles.tile([1, G4], F32R)
                nc.sync.dma_start(b1s[:], b1r[:])
            h0i = singles.tile([128, SLOT], F32R)
            nc.sync.dma_start(h0i[:], h0t0[:])
            h1i = singles.tile([128, SLOT], F32R)
            nc.sync.dma_start(h1i[:], h1t0[:])

            def mm(pb, lhsT, rhs, **kw):
                nc.tensor.matmul(pb, lhsT, rhs, **kw)

            st = {
                0: dict(prev=lambda j: h0i[:, j * BL:(j + 1) * BL],
                        c=None, htc=None),
                1: dict(prev=lambda j: h1i[:, j * BL:(j + 1) * BL],
                        c=None, htc=None),
            }
            cst0 = cstp.tile([BL, H], F32, tag="c0")
            nc.sync.dma_start(cst0[:], c00[:])
            st[0]["c"] = cst0
            cst1 = cstp.tile([BL, H], F32, tag="c1")
            nc.sync.dma_start(cst1[:], c10[:])
            st[1]["c"] = cst1
            gts = {}

            def step(layer, t):
                s = t % SPC
                if s == 0:
                    st[layer]["htc"] = htcp.tile(
                        [128, SPC * SLOT], F32R, tag=f"htc{layer}",
                        name=f"htc{layer}")
                htc = st[layer]["htc"]
                ps = psum.tile([128, G4], F32, tag=f"ps{layer}")
                prev = st[layer]["prev"]

                if layer == 0:
                    x_t = xp.tile([V, BL], F32R, tag="x")
                    nc.sync.dma_start(x_t[:], xt[t])
                else:
                    rg = rgp.tile([BL, G4], F32R, tag="rg")
                    gt = gts[t // SPC]
                    nc.sync.dma_start(rg[:], gt[BL * s:BL * (s + 1), :])
                ws = w0s if layer == 0 else w1hs
                for bank in range(NBANK):
                    pb = ps[0:BL, bank * BANK:(bank + 1) * BANK]
                    if layer == 0:
                        mm(pb, x_t[:], w0s[:, bank * BANK:bank * BANK + BANK],
                           start=True, stop=False)
                    else:
                        mm(pb, eye_s[:], rg[:, bank * BANK:(bank + 1) * BANK],
                           start=True, stop=False)
                    for j in range(KH):
                        off = ((1 + j) if layer == 0 else j) * G4 + bank * BANK
                        last = (j == KH - 1) and not (layer == 0 and has_b0)
                        mm(pb, prev(j), ws[:, off:off + BANK],
                           start=False, stop=last)
                    if layer == 0 and has_b0:
                        mm(pb, ones1[:, 0:BL],
                           b0s[:, bank * BANK:(bank + 1) * BANK],
                           start=False, stop=True)

                # activations: banks [g|i|f|o]
                tg = actp.tile([BL, BANK], F32, tag="tg")
                nc.scalar.activation(tg[:], ps[0:BL, 0:BANK], AF.Tanh)
                sifo = actp.tile([BL, 3 * BANK], F32, tag="sifo")
                nc.scalar.activation(sifo[:], ps[0:BL, BANK:4 * BANK],
                                     AF.Sigmoid)
                t1 = tmp.tile([BL, H], F32, tag="t1")
                nc.vector.tensor_mul(t1[:], sifo[:, 0:BANK], tg[:])
                t2 = tmp.tile([BL, H], F32, tag="t2")
                nc.vector.tensor_mul(t2[:], sifo[:, BANK:2 * BANK],
                                     st[layer]["c"][:])
                cnew = cstp.tile([BL, H], F32, tag=f"c{layer}")
                nc.vector.tensor_add(cnew[:], t1[:], t2[:])
                tc_ = tmp.tile([BL, H], F32, tag="tc")
                nc.scalar.activation(tc_[:], cnew[:], AF.Tanh)
                hnew = hp.tile([BL, H], F32, tag="h")
                nc.vector.tensor_mul(hnew[:], sifo[:, 2 * BANK:3 * BANK],
                                     tc_[:])
                st[layer]["c"] = cnew

                # transpose h into the tail of the o-bank, then to htc
                pt = ps[:, G4 - SLOT:G4]
                for j in range(KH):
                    nc.tensor.transpose(
                        pt[:, j * BL:(j + 1) * BL],
                        hnew[:, j * 128:(j + 1) * 128], eye_f[:])
                htc3 = htc[:].rearrange("p (j s2 b) -> p j s2 b",
                                        j=KH, s2=SPC, b=BL)[:, :, s]
                nc.vector.tensor_copy(htc3, pt[:])
                st[layer]["prev"] = (
                    lambda j, _h=htc, _s=s:
                    _h[:, j * SPC * BL + _s * BL:
                       j * SPC * BL + (_s + 1) * BL])

                if layer == 1:
                    nc.sync.dma_start(outs[:, t, :], hnew[:])
                if t == t_steps - 1:
                    nc.sync.dma_start(hf[layer], hnew[:])
                    nc.sync.dma_start(cf[layer], cnew[:])

            def b_chunk(mc):
                htc = st[0]["htc"]
                psb = psum.tile([128, G4], F32, tag="ps0")
                for bank in range(NBANK):
                    pb = psb[:, bank * BANK:(bank + 1) * BANK]
                    for j in range(KH):
                        off = j * G4 + bank * BANK
                        last = (j == KH - 1) and not has_b1
                        mm(pb, htc[:, j * SPC * BL:(j + 1) * SPC * BL],
                           w1xs[:, off:off + BANK],
                           start=(j == 0), stop=last)
                    if has_b1:
                        mm(pb, ones1[:], b1s[:, bank * BANK:(bank + 1) * BANK],
                           start=False, stop=True)
                gt = gtp.tile([128, G4], F32R, tag="gt")
                nc.vector.tensor_copy(gt[:], psb[:])
                gts[mc] = gt

            for mc in range(nch + 1):
                if mc < nch:
                    for s in range(SPC):
                        step(0, mc * SPC + s)
                    b_chunk(mc)
                if mc >= 1:
                    for s in range(SPC):
                        step(1, (mc - 1) * SPC + s)
                    gts.pop(mc - 1, None)

    nc.compile()
    return nc


def _get(t_steps: int, has_b0: bool, has_b1: bool):
    key = (t_steps, has_b0, has_b1)
    if key not in _BUILD_CACHE:
        _BUILD_CACHE[key] = _build(t_steps, has_b0, has_b1)
    return _BUILD_CACHE[key]


def _hT_chunks(hmat: np.ndarray) -> np.ndarray:
    """(BL, H) -> (128, KH*BL) stationary layout (chunk-major columns)."""
    return np.ascontiguousarray(
        hmat.T.reshape(KH, 128, BL).transpose(1, 0, 2).reshape(128, KH * BL))


def kernel(x, h, c, Wx0, Wh0, b0, Wx1, Wh1, b1, t_steps: int = T):
    x = np.asarray(x, np.float32)
    h = np.asarray(h, np.float32)
    c = np.asarray(c, np.float32)
    Wx0 = np.asarray(Wx0, np.float32)
    Wh0 = np.asarray(Wh0, np.float32)
    b0 = np.asarray(b0, np.float32)
    Wx1 = np.asarray(Wx1, np.float32)
    Wh1 = np.asarray(Wh1, np.float32)
    b1 = np.asarray(b1, np.float32)

    has_b0 = bool(np.any(b0))
    has_b1 = bool(np.any(b1))
    nc = _get(t_steps, has_b0, has_b1)

    def _gperm(w):
        # reorder gate column groups [i f g o] -> [g i f o]
        i_, f_, g_, o_ = np.split(w, 4, axis=-1)
        return np.concatenate([g_, i_, f_, o_], axis=-1)

    Wx0, Wh0, Wx1, Wh1 = map(_gperm, (Wx0, Wh0, Wx1, Wh1))
    b0, b1 = _gperm(b0), _gperm(b1)
    w0cat = np.concatenate(
        [Wx0[None], Wh0.reshape(KH, 128, G4)], axis=0)  # (5,128,G4)
    w0_in = np.ascontiguousarray(
        w0cat.transpose(1, 0, 2).reshape(V, 5 * G4))
    w1x_in = np.ascontiguousarray(
        Wx1.reshape(KH, 128, G4).transpose(1, 0, 2).reshape(128, KH * G4))
    w1h_in = np.ascontiguousarray(
        Wh1.reshape(KH, 128, G4).transpose(1, 0, 2).reshape(128, KH * G4))
    eye_in = np.eye(BL, dtype=np.float32)
    b0_in = np.ascontiguousarray(b0[None])
    b1_in = np.ascontiguousarray(b1[None])

    in_maps = []
    for i in range(NCORES):
        s = slice(i * BL, (i + 1) * BL)
        in_maps.append({
            "xt": np.ascontiguousarray(x[s, :t_steps].transpose(1, 2, 0)),
            "w0": w0_in, "w1x": w1x_in, "w1h": w1h_in,
            "eye": eye_in, "b0r": b0_in, "b1r": b1_in,
            "onesr": np.ones((1, 128), np.float32),
            "h0t0": _hT_chunks(h[0, s]),
            "h1t0": _hT_chunks(h[1, s]),
            "c00": np.ascontiguousarray(c[0, s]),
            "c10": np.ascontiguousarray(c[1, s]),
        })

    res = run_bass_kernel_spmd(nc, in_maps, core_ids=list(range(NCORES)),
                               **RUN_KWARGS)
    kernel.last_results = res

    outs = np.empty((B, t_steps, H), np.float32)
    h_f = np.empty((2, B, H), np.float32)
    c_f = np.empty((2, B, H), np.float32)
    for i in range(NCORES):
        s = slice(i * BL, (i + 1) * BL)
        outs[s] = res.results[i]["outs"]
        h_f[:, s] = res.results[i]["hf"]
        c_f[:, s] = res.results[i]["cf"]
    return outs, h_f, c_f


# revision 14
# speedup vs baseline: 1.9823x; 1.9110x over previous
"""Trainium2 Bass kernel for a 2-layer LSTM (B=128, T=512, V=128, H=512).

Data-parallel over batch (8 cores, B_local=16). Wavefront schedule:
layer-0 steps of time-chunk mc, then the batched H0@Wx1 transform for
that chunk, then layer-1 steps of chunk mc-1 — all interleaved so each
layer's serial activation tail hides under the other layer's matmuls.
Gate matmuls run as float32r (single-pass PE). All staging in SBUF —
no DRAM bounce buffers.

Per step (banks ordered [g|i|f|o]):
  psum(16,2048) = x_t^T @ Wx0 (L0) or G1_t via k=16 identity (L1),
                  += 4 h-chunk matmuls against Wh (moving weights)
  tanh/sigmoid on ACT, c/h update on DVE, h transposed back via PE
  transpose into the tail of the o-bank psum, copied to the chunk's
  hT tile (stationary for the next step and the batched transform).
"""
import sys

import numpy as np

for _p in ("/opt/trn_rl_repo", "/root/.axon_site/_ro/trn_rl_repo"):
    if _p not in sys.path:
        sys.path.insert(0, _p)

import concourse.bacc as bacc
import concourse.mybir as mybir
import concourse.tile as tile
from concourse.bass_utils import run_bass_kernel_spmd

B, T, V, H = 128, 512, 128, 512
G4 = 4 * H            # 2048 gate columns, order [g|i|f|o]
NCORES = 8
BL = B // NCORES      # 16 batch rows per core
KH = H // 128         # 4 contraction chunks for H
NBANK = 4
BANK = G4 // NBANK    # 512 = one psum bank (fp32)
SPC = 8               # time steps per chunk
SLOT = KH * BL        # 64 cols of hT per step

F32 = mybir.dt.float32
F32R = mybir.dt.float32r
AF = mybir.ActivationFunctionType

RUN_KWARGS: dict = {}
_BUILD_CACHE: dict = {}


def _build(t_steps: int, has_b0: bool, has_b1: bool):
    assert t_steps % SPC == 0
    nch = t_steps // SPC
    nc = bacc.Bacc("TRN2", target_bir_lowering=False, debug=False,
                   num_devices=NCORES)

    xt = nc.dram_tensor("xt", [t_steps, V, BL], F32R, kind="ExternalInput")
    w0 = nc.dram_tensor("w0", [V, 5 * G4], F32R, kind="ExternalInput")
    w1x = nc.dram_tensor("w1x", [128, KH * G4], F32R, kind="ExternalInput")
    w1h = nc.dram_tensor("w1h", [128, KH * G4], F32R, kind="ExternalInput")
    eye = nc.dram_tensor("eye", [BL, BL], F32R, kind="ExternalInput")
    b0r = nc.dram_tensor("b0r", [1, G4], F32R, kind="ExternalInput")
    b1r = nc.dram_tensor("b1r", [1, G4], F32R, kind="ExternalInput")
    onesr = nc.dram_tensor("onesr", [1, 128], F32R, kind="ExternalInput")
    h0t0 = nc.dram_tensor("h0t0", [128, SLOT], F32R, kind="ExternalInput")
    h1t0 = nc.dram_tensor("h1t0", [128, SLOT], F32R, kind="ExternalInput")
    c00 = nc.dram_tensor("c00", [BL, H], F32, kind="ExternalInput")
    c10 = nc.dram_tensor("c10", [BL, H], F32, kind="ExternalInput")

    outs = nc.dram_tensor("outs", [BL, t_steps, H], F32, kind="ExternalOutput")
    hf = nc.dram_tensor("hf", [2, BL, H], F32, kind="ExternalOutput")
    cf = nc.dram_tensor("cf", [2, BL, H], F32, kind="ExternalOutput")

    with tile.TileContext(nc) as tc:
        with (
            tc.tile_pool(name="singles", bufs=1) as singles,
            tc.tile_pool(name="xp", bufs=3) as xp,
            tc.tile_pool(name="rgp", bufs=2) as rgp,
            tc.tile_pool(name="psum", bufs=1, space="PSUM") as psum,
            tc.tile_pool(name="actp", bufs=2) as actp,
            tc.tile_pool(name="tmp", bufs=3) as tmp,
            tc.tile_pool(name="cstp", bufs=2) as cstp,
            tc.tile_pool(name="hp", bufs=3) as hp,
            tc.tile_pool(name="htcp", bufs=2) as htcp,
            tc.tile_pool(name="gtp", bufs=2) as gtp,
        ):
            w0s = singles.tile([V, 5 * G4], F32R)
            nc.sync.dma_start(w0s[:], w0[:])
            w1xs = singles.tile([128, KH * G4], F32R)
            nc.sync.dma_start(w1xs[:], w1x[:])
            w1hs = singles.tile([128, KH * G4], F32R)
            nc.sync.dma_start(w1hs[:], w1h[:])
            eye_s = singles.tile([BL, BL], F32R)
            nc.sync.dma_start(eye_s[:], eye[:])
            eye_f = singles.tile([BL, BL], F32)
            nc.sync.dma_start(eye_f[:], eye[:].bitcast(F32))
            ones1 = None
            if has_b0 or has_b1:
                ones1 = singles.tile([1, 128], F32R)
                nc.sync.dma_start(ones1[:], onesr[:])
            b0s = b1s = None
            if has_b0:
                b0s = singles.tile([1, G4], F32R)
                nc.sync.dma_start(b0s[:], b0r[:])
            if has_b1:
                b1s = singles.tile([1, G4], F32R)
                nc.sync.dma_start(b1s[:], b1r[:])
            h0i = singles.tile([128, SLOT], F32R)
            nc.sync.dma_start(h0i[:], h0t0[:])
            h1i = singles.tile([128, SLOT], F32R)
            nc.sync.dma_start(h1i[:], h1t0[:])

            def mm(pb, lhsT, rhs, **kw):
                nc.tensor.matmul(pb, lhsT, rhs, **kw)

            st = {
                0: dict(prev=lambda j: h0i[:, j * BL:(j + 1) * BL],
                        c=None, htc=None),
                1: dict(prev=lambda j: h1i[:, j * BL:(j + 1) * BL],
                        c=None, htc=None),
            }
            cst0 = cstp.tile([BL, H], F32, tag="c0")
            nc.sync.dma_start(cst0[:], c00[:])
            st[0]["c"] = cst0
            cst1 = cstp.tile([BL, H], F32, tag="c1")
            nc.sync.dma_start(cst1[:], c10[:])
            st[1]["c"] = cst1
            gts = {}

            def step(layer, t):
                s = t % SPC
                if s == 0:
                    st[layer]["htc"] = htcp.tile(
                        [128, SPC * SLOT], F32R, tag=f"htc{layer}",
                        name=f"htc{layer}")
                htc = st[layer]["htc"]
                ps = psum.tile([128, G4], F32, tag=f"ps{layer}")
                prev = st[layer]["prev"]

                if layer == 0:
                    x_t = xp.tile([V, BL], F32R, tag="x")
                    nc.sync.dma_start(x_t[:], xt[t])
                else:
                    rg = rgp.tile([BL, G4], F32R, tag="rg")
                    gt = gts[t // SPC]
                    nc.sync.dma_start(rg[:], gt[BL * s:BL * (s + 1), :])
                ws = w0s if layer == 0 else w1hs
                for bank in range(NBANK):
                    pb = ps[0:BL, bank * BANK:(bank + 1) * BANK]
                    if layer == 0:
                        mm(pb, x_t[:], w0s[:, bank * BANK:bank * BANK + BANK],
                           start=True, stop=False)
                    else:
                        mm(pb, eye_s[:], rg[:, bank * BANK:(bank + 1) * BANK],
                           start=True, stop=False)
                    for j in range(KH):
                        off = ((1 + j) if layer == 0 else j) * G4 + bank * BANK
                        last = (j == KH - 1) and not (layer == 0 and has_b0)
                        mm(pb, prev(j), ws[:, off:off + BANK],
                           start=False, stop=last)
                    if layer == 0 and has_b0:
                        mm(pb, ones1[:, 0:BL],
                           b0s[:, bank * BANK:(bank + 1) * BANK],
                           start=False, stop=True)

                # activations: banks [g|i|f|o]
                tg = actp.tile([BL, BANK], F32, tag="tg")
                nc.scalar.activation(tg[:], ps[0:BL, 0:BANK], AF.Tanh)
                sifo = actp.tile([BL, 3 * BANK], F32, tag="sifo")
                nc.scalar.activation(sifo[:], ps[0:BL, BANK:4 * BANK],
                                     AF.Sigmoid)
                t1 = tmp.tile([BL, H], F32, tag="t1")
                nc.vector.tensor_mul(t1[:], sifo[:, 0:BANK], tg[:])
                t2 = tmp.tile([BL, H], F32, tag="t2")
                nc.vector.tensor_mul(t2[:], sifo[:, BANK:2 * BANK],
                                     st[layer]["c"][:])
                cnew = cstp.tile([BL, H], F32, tag=f"c{layer}")
                nc.vector.tensor_add(cnew[:], t1[:], t2[:])
                tc_ = tmp.tile([BL, H], F32, tag="tc")
                nc.scalar.activation(tc_[:], cnew[:], AF.Tanh)
                hnew = hp.tile([BL, H], F32, tag="h")
                nc.vector.tensor_mul(hnew[:], sifo[:, 2 * BANK:3 * BANK],
                                     tc_[:])
                st[layer]["c"] = cnew

                # transpose h into the tail of the o-bank, then to htc
                pt = ps[:, G4 - SLOT:G4]
                for j in range(KH):
                    nc.tensor.transpose(
                        pt[:, j * BL:(j + 1) * BL],
                        hnew[:, j * 128:(j + 1) * 128], eye_f[:])
                htc3 = htc[:].rearrange("p (j s2 b) -> p j s2 b",
                                        j=KH, s2=SPC, b=BL)[:, :, s]
                nc.vector.tensor_copy(htc3, pt[:])
                st[layer]["prev"] = (
                    lambda j, _h=htc, _s=s:
                    _h[:, j * SPC * BL + _s * BL:
                       j * SPC * BL + (_s + 1) * BL])

                if layer == 1:
                    nc.sync.dma_start(outs[:, t, :], hnew[:])
                if t == t_steps - 1:
                    nc.sync.dma_start(hf[layer], hnew[:])
                    nc.sync.dma_start(cf[layer], cnew[:])

            def b_chunk(mc):
                htc = st[0]["htc"]
                psb = psum.tile([128, G4], F32, tag="ps0")
                for bank in range(NBANK):
                    pb = psb[:, bank * BANK:(bank + 1) * BANK]
                    for j in range(KH):
                        off = j * G4 + bank * BANK
                        last = (j == KH - 1) and not has_b1
                        mm(pb, htc[:, j * SPC * BL:(j + 1) * SPC * BL],
                           w1xs[:, off:off + BANK],
                           start=(j == 0), stop=last)
                    if has_b1:
                        mm(pb, ones1[:], b1s[:, bank * BANK:(bank + 1) * BANK],
                           start=False, stop=True)
                gt = gtp.tile([128, G4], F32R, tag="gt")
                nc.vector.tensor_copy(gt[:], psb[:])
                gts[mc] = gt

            for mc in range(nch + 1):
                for s in range(SPC):
                    if mc < nch:
                        step(0, mc * SPC + s)
                    if mc >= 1:
                        step(1, (mc - 1) * SPC + s)
                if mc < nch:
                    b_chunk(mc)
                if mc >= 1:
                    gts.pop(mc - 1, None)

    nc.compile()
    return nc


def _get(t_steps: int, has_b0: bool, has_b1: bool):
    key = (t_steps, has_b0, has_b1)
    if key not in _BUILD_CACHE:
        _BUILD_CACHE[key] = _build(t_steps, has_b0, has_b1)
    return _BUILD_CACHE[key]


def _hT_chunks(hmat: np.ndarray) -> np.ndarray:
    """(BL, H) -> (128, KH*BL) stationary layout (chunk-major columns)."""
    return np.ascontiguousarray(
        hmat.T.reshape(KH, 128, BL).transpose(1, 0, 2).reshape(128, KH * BL))


def kernel(x, h, c, Wx0, Wh0, b0, Wx1, Wh1, b1, t_steps: int = T):
    x = np.asarray(x, np.float32)
    h = np.asarray(h, np.float32)
    c = np.asarray(c, np.float32)
    Wx0 = np.asarray(Wx0, np.float32)
    Wh0 = np.asarray(Wh0, np.float32)
    b0 = np.asarray(b0, np.float32)
    Wx1 = np.asarray(Wx1, np.float32)
    Wh1 = np.asarray(Wh1, np.float32)
    b1 = np.asarray(b1, np.float32)

    has_b0 = bool(np.any(b0))
    has_b1 = bool(np.any(b1))
    nc = _get(t_steps, has_b0, has_b1)

    def _gperm(w):
        # reorder gate column groups [i f g o] -> [g i f o]
        i_, f_, g_, o_ = np.split(w, 4, axis=-1)
        return np.concatenate([g_, i_, f_, o_], axis=-1)

    Wx0, Wh0, Wx1, Wh1 = map(_gperm, (Wx0, Wh0, Wx1, Wh1))
    b0, b1 = _gperm(b0), _gperm(b1)
    w0cat = np.concatenate(
        [Wx0[None], Wh0.reshape(KH, 128, G4)], axis=0)  # (5,128,G4)
    w0_in = np.ascontiguousarray(
        w0cat.transpose(1, 0, 2).reshape(V, 5 * G4))
    w1x_in = np.ascontiguousarray(
        Wx1.reshape(KH, 128, G4).transpose(1, 0, 2).reshape(128, KH * G4))
    w1h_in = np.ascontiguousarray(
        Wh1.reshape(KH, 128, G4).transpose(1, 0, 2).reshape(128, KH * G4))
    eye_in = np.eye(BL, dtype=np.float32)
    b0_in = np.ascontiguousarray(b0[None])
    b1_in = np.ascontiguousarray(b1[None])

    in_maps = []
    for i in range(NCORES):
        s = slice(i * BL, (i + 1) * BL)
        in_maps.append({
            "xt": np.ascontiguousarray(x[s, :t_steps].transpose(1, 2, 0)),
            "w0": w0_in, "w1x": w1x_in, "w1h": w1h_in,
            "eye": eye_in, "b0r": b0_in, "b1r": b1_in,
            "onesr": np.ones((1, 128), np.float32),
            "h0t0": _hT_chunks(h[0, s]),
            "h1t0": _hT_chunks(h[1, s]),
            "c00": np.ascontiguousarray(c[0, s]),
            "c10": np.ascontiguousarray(c[1, s]),
        })

    res = run_bass_kernel_spmd(nc, in_maps, core_ids=list(range(NCORES)),
                               **RUN_KWARGS)
    kernel.last_results = res

    outs = np.empty((B, t_steps, H), np.float32)
    h_f = np.empty((2, B, H), np.float32)
    c_f = np.empty((2, B, H), np.float32)
    for i in range(NCORES):
        s = slice(i * BL, (i + 1) * BL)
        outs[s] = res.results[i]["outs"]
        h_f[:, s] = res.results[i]["hf"]
        c_f[:, s] = res.results[i]["cf"]
    return outs, h_f, c_f


# revision 15
# speedup vs baseline: 2.1061x; 1.0624x over previous
"""Trainium2 Bass kernel for a 2-layer LSTM (B=128, T=512, V=128, H=512).

Data-parallel over batch (8 cores, B_local=16). Wavefront schedule:
layer-0 steps of time-chunk mc, then the batched H0@Wx1 transform for
that chunk, then layer-1 steps of chunk mc-1 — all interleaved so each
layer's serial activation tail hides under the other layer's matmuls.
Gate matmuls run as float32r (single-pass PE). All staging in SBUF —
no DRAM bounce buffers.

Per step (banks ordered [g|i|f|o]):
  psum(16,2048) = x_t^T @ Wx0 (L0) or G1_t via k=16 identity (L1),
                  += 4 h-chunk matmuls against Wh (moving weights)
  tanh/sigmoid on ACT, c/h update on DVE, h transposed back via PE
  transpose into the tail of the o-bank psum, copied to the chunk's
  hT tile (stationary for the next step and the batched transform).
"""
import sys

import numpy as np

for _p in ("/opt/trn_rl_repo", "/root/.axon_site/_ro/trn_rl_repo"):
    if _p not in sys.path:
        sys.path.insert(0, _p)

import concourse.bacc as bacc
import concourse.mybir as mybir
import concourse.tile as tile
from concourse.bass_utils import run_bass_kernel_spmd

B, T, V, H = 128, 512, 128, 512
G4 = 4 * H            # 2048 gate columns, order [g|i|f|o]
NCORES = 8
BL = B // NCORES      # 16 batch rows per core
KH = H // 128         # 4 contraction chunks for H
NBANK = 4
BANK = G4 // NBANK    # 512 = one psum bank (fp32)
SPC = 8               # time steps per chunk
SLOT = KH * BL        # 64 cols of hT per step

F32 = mybir.dt.float32
F32R = mybir.dt.float32r
AF = mybir.ActivationFunctionType

RUN_KWARGS: dict = {}
_BUILD_CACHE: dict = {}


def _build(t_steps: int, has_b0: bool, has_b1: bool):
    assert t_steps % SPC == 0
    nch = t_steps // SPC
    nc = bacc.Bacc("TRN2", target_bir_lowering=False, debug=False,
                   num_devices=NCORES)

    xt = nc.dram_tensor("xt", [t_steps, V, BL], F32R, kind="ExternalInput")
    w0 = nc.dram_tensor("w0", [V, 5 * G4], F32R, kind="ExternalInput")
    w1x = nc.dram_tensor("w1x", [128, KH * G4], F32R, kind="ExternalInput")
    w1h = nc.dram_tensor("w1h", [128, KH * G4], F32R, kind="ExternalInput")
    eye = nc.dram_tensor("eye", [BL, BL], F32R, kind="ExternalInput")
    b0r = nc.dram_tensor("b0r", [1, G4], F32R, kind="ExternalInput")
    b1r = nc.dram_tensor("b1r", [1, G4], F32R, kind="ExternalInput")
    onesr = nc.dram_tensor("onesr", [1, 128], F32R, kind="ExternalInput")
    h0t0 = nc.dram_tensor("h0t0", [128, SLOT], F32R, kind="ExternalInput")
    h1t0 = nc.dram_tensor("h1t0", [128, SLOT], F32R, kind="ExternalInput")
    c00 = nc.dram_tensor("c00", [BL, H], F32, kind="ExternalInput")
    c10 = nc.dram_tensor("c10", [BL, H], F32, kind="ExternalInput")

    outs = nc.dram_tensor("outs", [BL, t_steps, H], F32, kind="ExternalOutput")
    hf = nc.dram_tensor("hf", [2, BL, H], F32, kind="ExternalOutput")
    cf = nc.dram_tensor("cf", [2, BL, H], F32, kind="ExternalOutput")

    with tile.TileContext(nc) as tc:
        with (
            tc.tile_pool(name="singles", bufs=1) as singles,
            tc.tile_pool(name="xp", bufs=3) as xp,
            tc.tile_pool(name="rgp", bufs=2) as rgp,
            tc.tile_pool(name="psum", bufs=1, space="PSUM") as psum,
            tc.tile_pool(name="actp", bufs=2) as actp,
            tc.tile_pool(name="tmp", bufs=3) as tmp,
            tc.tile_pool(name="cstp", bufs=2) as cstp,
            tc.tile_pool(name="hp", bufs=3) as hp,
            tc.tile_pool(name="htcp", bufs=2) as htcp,
            tc.tile_pool(name="gtp", bufs=2) as gtp,
        ):
            w0s = singles.tile([V, 5 * G4], F32R)
            nc.sync.dma_start(w0s[:], w0[:])
            w1xs = singles.tile([128, KH * G4], F32R)
            nc.sync.dma_start(w1xs[:], w1x[:])
            w1hs = singles.tile([128, KH * G4], F32R)
            nc.sync.dma_start(w1hs[:], w1h[:])
            eye_s = singles.tile([BL, BL], F32R)
            nc.sync.dma_start(eye_s[:], eye[:])
            eye_f = singles.tile([BL, BL], F32)
            nc.sync.dma_start(eye_f[:], eye[:].bitcast(F32))
            ones1 = None
            if has_b0 or has_b1:
                ones1 = singles.tile([1, 128], F32R)
                nc.sync.dma_start(ones1[:], onesr[:])
            b0s = b1s = None
            if has_b0:
                b0s = singles.tile([1, G4], F32R)
                nc.sync.dma_start(b0s[:], b0r[:])
            if has_b1:
                b1s = singles.tile([1, G4], F32R)
                nc.sync.dma_start(b1s[:], b1r[:])
            h0i = singles.tile([128, SLOT], F32R)
            nc.sync.dma_start(h0i[:], h0t0[:])
            h1i = singles.tile([128, SLOT], F32R)
            nc.sync.dma_start(h1i[:], h1t0[:])

            def mm(pb, lhsT, rhs, **kw):
                nc.tensor.matmul(pb, lhsT, rhs, **kw)

            st = {
                0: dict(prev=lambda j: h0i[:, j * BL:(j + 1) * BL],
                        c=None, htc=None),
                1: dict(prev=lambda j: h1i[:, j * BL:(j + 1) * BL],
                        c=None, htc=None),
            }
            cst0 = cstp.tile([BL, H], F32, tag="c0")
            nc.sync.dma_start(cst0[:], c00[:])
            st[0]["c"] = cst0
            cst1 = cstp.tile([BL, H], F32, tag="c1")
            nc.sync.dma_start(cst1[:], c10[:])
            st[1]["c"] = cst1
            gts = {}

            def step(layer, t):
                s = t % SPC
                if s == 0:
                    st[layer]["htc"] = htcp.tile(
                        [128, SPC * SLOT], F32R, tag=f"htc{layer}",
                        name=f"htc{layer}")
                htc = st[layer]["htc"]
                ps = psum.tile([128, G4], F32, tag=f"ps{layer}")
                prev = st[layer]["prev"]

                if layer == 0:
                    x_t = xp.tile([V, BL], F32R, tag="x")
                    nc.sync.dma_start(x_t[:], xt[t])
                else:
                    rg = rgp.tile([BL, G4], F32R, tag="rg")
                    gt = gts[t // SPC]
                    nc.sync.dma_start(rg[:], gt[BL * s:BL * (s + 1), :])
                ws = w0s if layer == 0 else w1hs
                for bank in range(NBANK):
                    pb = ps[0:BL, bank * BANK:(bank + 1) * BANK]
                    if layer == 0:
                        mm(pb, x_t[:], w0s[:, bank * BANK:bank * BANK + BANK],
                           start=True, stop=False)
                    else:
                        mm(pb, eye_s[:], rg[:, bank * BANK:(bank + 1) * BANK],
                           start=True, stop=False)
                    for j in range(KH):
                        off = ((1 + j) if layer == 0 else j) * G4 + bank * BANK
                        last = (j == KH - 1) and not (layer == 0 and has_b0)
                        mm(pb, prev(j), ws[:, off:off + BANK],
                           start=False, stop=last)
                    if layer == 0 and has_b0:
                        mm(pb, ones1[:, 0:BL],
                           b0s[:, bank * BANK:(bank + 1) * BANK],
                           start=False, stop=True)

                # activations: banks [g|i|f|o]
                tg = actp.tile([BL, BANK], F32, tag="tg")
                nc.scalar.activation(tg[:], ps[0:BL, 0:BANK], AF.Tanh)
                sif = actp.tile([BL, 2 * BANK], F32, tag="sif")
                nc.scalar.activation(sif[:], ps[0:BL, BANK:3 * BANK],
                                     AF.Sigmoid)
                t1 = tmp.tile([BL, H], F32, tag="t1")
                nc.vector.tensor_mul(t1[:], sif[:, 0:BANK], tg[:])
                t2 = tmp.tile([BL, H], F32, tag="t2")
                nc.vector.tensor_mul(t2[:], sif[:, BANK:2 * BANK],
                                     st[layer]["c"][:])
                cnew = cstp.tile([BL, H], F32, tag=f"c{layer}")
                nc.vector.tensor_add(cnew[:], t1[:], t2[:])
                so = actp.tile([BL, BANK], F32, tag="so")
                nc.scalar.activation(so[:], ps[0:BL, 3 * BANK:4 * BANK],
                                     AF.Sigmoid)
                tc_ = tmp.tile([BL, H], F32, tag="tc")
                nc.scalar.activation(tc_[:], cnew[:], AF.Tanh)
                hnew = hp.tile([BL, H], F32, tag="h")
                nc.vector.tensor_mul(hnew[:], so[:], tc_[:])
                st[layer]["c"] = cnew

                # transpose h into the tail of the o-bank, then to htc
                pt = ps[:, G4 - SLOT:G4]
                for j in range(KH):
                    nc.tensor.transpose(
                        pt[:, j * BL:(j + 1) * BL],
                        hnew[:, j * 128:(j + 1) * 128], eye_f[:])
                htc3 = htc[:].rearrange("p (j s2 b) -> p j s2 b",
                                        j=KH, s2=SPC, b=BL)[:, :, s]
                nc.vector.tensor_copy(htc3, pt[:])
                st[layer]["prev"] = (
                    lambda j, _h=htc, _s=s:
                    _h[:, j * SPC * BL + _s * BL:
                       j * SPC * BL + (_s + 1) * BL])

                if layer == 1:
                    nc.sync.dma_start(outs[:, t, :], hnew[:])
                if t == t_steps - 1:
                    nc.sync.dma_start(hf[layer], hnew[:])
                    nc.sync.dma_start(cf[layer], cnew[:])

            def b_chunk(mc):
                htc = st[0]["htc"]
                psb = psum.tile([128, G4], F32, tag="ps0")
                for bank in range(NBANK):
                    pb = psb[:, bank * BANK:(bank + 1) * BANK]
                    for j in range(KH):
                        off = j * G4 + bank * BANK
                        last = (j == KH - 1) and not has_b1
                        mm(pb, htc[:, j * SPC * BL:(j + 1) * SPC * BL],
                           w1xs[:, off:off + BANK],
                           start=(j == 0), stop=last)
                    if has_b1:
                        mm(pb, ones1[:], b1s[:, bank * BANK:(bank + 1) * BANK],
                           start=False, stop=True)
                gt = gtp.tile([128, G4], F32R, tag="gt")
                nc.vector.tensor_copy(gt[:], psb[:])
                gts[mc] = gt

            for mc in range(nch + 1):
                for s in range(SPC):
                    if mc < nch:
                        step(0, mc * SPC + s)
                    if mc >= 1:
                        step(1, (mc - 1) * SPC + s)
                if mc < nch:
                    b_chunk(mc)
                if mc >= 1:
                    gts.pop(mc - 1, None)

    nc.compile()
    return nc


def _get(t_steps: int, has_b0: bool, has_b1: bool):
    key = (t_steps, has_b0, has_b1)
    if key not in _BUILD_CACHE:
        _BUILD_CACHE[key] = _build(t_steps, has_b0, has_b1)
    return _BUILD_CACHE[key]


def _hT_chunks(hmat: np.ndarray) -> np.ndarray:
    """(BL, H) -> (128, KH*BL) stationary layout (chunk-major columns)."""
    return np.ascontiguousarray(
        hmat.T.reshape(KH, 128, BL).transpose(1, 0, 2).reshape(128, KH * BL))


def kernel(x, h, c, Wx0, Wh0, b0, Wx1, Wh1, b1, t_steps: int = T):
    x = np.asarray(x, np.float32)
    h = np.asarray(h, np.float32)
    c = np.asarray(c, np.float32)
    Wx0 = np.asarray(Wx0, np.float32)
    Wh0 = np.asarray(Wh0, np.float32)
    b0 = np.asarray(b0, np.float32)
    Wx1 = np.asarray(Wx1, np.float32)
    Wh1 = np.asarray(Wh1, np.float32)
    b1 = np.asarray(b1, np.float32)

    has_b0 = bool(np.any(b0))
    has_b1 = bool(np.any(b1))
    nc = _get(t_steps, has_b0, has_b1)

    def _gperm(w):
        # reorder gate column groups [i f g o] -> [g i f o]
        i_, f_, g_, o_ = np.split(w, 4, axis=-1)
        return np.concatenate([g_, i_, f_, o_], axis=-1)

    Wx0, Wh0, Wx1, Wh1 = map(_gperm, (Wx0, Wh0, Wx1, Wh1))
    b0, b1 = _gperm(b0), _gperm(b1)
    w0cat = np.concatenate(
        [Wx0[None], Wh0.reshape(KH, 128, G4)], axis=0)  # (5,128,G4)
    w0_in = np.ascontiguousarray(
        w0cat.transpose(1, 0, 2).reshape(V, 5 * G4))
    w1x_in = np.ascontiguousarray(
        Wx1.reshape(KH, 128, G4).transpose(1, 0, 2).reshape(128, KH * G4))
    w1h_in = np.ascontiguousarray(
        Wh1.reshape(KH, 128, G4).transpose(1, 0, 2).reshape(128, KH * G4))
    eye_in = np.eye(BL, dtype=np.float32)
    b0_in = np.ascontiguousarray(b0[None])
    b1_in = np.ascontiguousarray(b1[None])

    in_maps = []
    for i in range(NCORES):
        s = slice(i * BL, (i + 1) * BL)
        in_maps.append({
            "xt": np.ascontiguousarray(x[s, :t_steps].transpose(1, 2, 0)),
            "w0": w0_in, "w1x": w1x_in, "w1h": w1h_in,
            "eye": eye_in, "b0r": b0_in, "b1r": b1_in,
            "onesr": np.ones((1, 128), np.float32),
            "h0t0": _hT_chunks(h[0, s]),
            "h1t0": _hT_chunks(h[1, s]),
            "c00": np.ascontiguousarray(c[0, s]),
            "c10": np.ascontiguousarray(c[1, s]),
        })

    res = run_bass_kernel_spmd(nc, in_maps, core_ids=list(range(NCORES)),
                               **RUN_KWARGS)
    kernel.last_results = res

    outs = np.empty((B, t_steps, H), np.float32)
    h_f = np.empty((2, B, H), np.float32)
    c_f = np.empty((2, B, H), np.float32)
    for i in range(NCORES):
        s = slice(i * BL, (i + 1) * BL)
        outs[s] = res.results[i]["outs"]
        h_f[:, s] = res.results[i]["hf"]
        c_f[:, s] = res.results[i]["cf"]
    return outs, h_f, c_f
